# revision 18
# baseline (speedup 1.0000x reference)
"""3x3 NMS (maxpool + threshold + border) kernel for Trainium2, 8 NeuronCores.

Strategy:
  - Pure data parallel: 16 images -> 2 images per core on 8 cores.
  - Both images of a core are packed into the partition dim: partition
    p = img*64 + blk holds R=24 image rows (+1 halo row each side, from
    a host-zero-padded copy), so the row-halo DMA overhead is 26/24.
  - The image is split into NT column tiles (2-col halos). The first and
    last tiles are narrow to shrink the pipeline fill / drain on the
    critical path.
  - Host stages the input TILE-MAJOR: for each tile, a contiguous
    [128, 26, WT] block (halo rows/cols duplicated), so every DMA
    descriptor is one full partition worth (26*WT*4 B) and the 16 DMA
    engines run at full rate. The mask output is likewise tile-major
    [128, 24, V+2] contiguous; host reassembles/strips junk columns.
  - Per tile, 2 vector-engine passes:
      1. v1 = max(x_up, x_dn)            (stock tensor_tensor, 1 el/cyc)
      2. mask = ANT_NMS_FUSED(v1, x): a custom DVE uop computing
         vm = max(v1, x), the horizontal sliding 3-max of vm, the 0.6
         clamp, and the compare  mask[c] = (x[c] >= max(vm[c-1..c+1],
         0.6))  in ONE 1-elem/cycle pass, using delay-chain captures for
         the previous-element taps (incl. a 1-delayed x for the center).
    x >= max(window incl. center, thr) is exactly
    (x == maxpool3x3(x)) & (x >= thr): bit-exact, no FP hazards.
  - Host: zero 10px border, np.nonzero -> (y, x) rows, exactly matching
    jnp.nonzero order (batch-major, then row, then col).
"""

import os
import sys

sys.path.insert(0, "/opt/trn_rl_repo")

import numpy as np

B, C, H, W = 16, 1, 1536, 1536
HP = H + 2                    # padded rows
N_CORES = 8
B_PER = B // N_CORES          # images per core
R = 24                        # rows per partition (2 imgs * 64 blocks = 128)
NB = H // R                   # row blocks per image (64)
PAD = 2                       # column halo on each side
REP_THR = 0.6

# column tile widths: progressive ramp so the DVE can start while the DMA
# stream is still ramping, without ever outrunning it
WIDTHS = [32, 64, 96, 144, 192, 240, 256, 256, 256]
assert sum(WIDTHS) == W
NT = len(WIDTHS)
# tile t covers mask cols [C0[t], C0[t]+WIDTHS[t]), reads [cs, ce)
C0 = [sum(WIDTHS[:i]) for i in range(NT)]

_CACHE = {}
LAST_RESULTS = None


def _tile_geom(t):
    c0, v = C0[t], WIDTHS[t]
    cs = max(c0 - PAD, 0)
    ce = min(c0 + v + PAD, W)
    return c0, v, cs, ce - cs


def _build_program():
    import concourse.bass as bass
    import concourse.bacc as bacc
    import concourse.mybir as mybir
    from concourse.tile import TileContext

    f32 = mybir.dt.float32
    u8 = mybir.dt.uint8
    MAX = mybir.AluOpType.max

    from concourse.dve_ops import DveOp, OPS, _COMPILE_CACHE
    from concourse.dve_spec import Spec, Src0, Src1, C0 as DC0, maxx, lower
    from concourse.dve_uop import (
        DveOpSpec, InpSel, OutSel, OutPath, AluInp, DelayInp, AluOp,
    )
    from concourse.dve_ops import get_dve_sub_opcode


    def _mk_fused_uop(base_uop):
        """One fused NMS pass. Stream pos i carries v1[i] (src0) and x[i]
        (src1); output at pos i is mask for the column one behind:
        out(i) = (x(i-1) >= max(0.6, vm(i-2), vm(i-1), vm(i))) with
        vm(j) = max(v1(j), x(j)).

        Delay chains (v3 has 6): 0 = v1 in, 1 = C0, 2 = x in,
        3 = x delayed one element, 4 = vm(i-1) tap, 5 = m1(i-1) tap.
        """
        u = base_uop  # copy of a lowered stock uop: keeps FSM/trigger/ctrl
        for i in range(len(u.inp)):
            u.inp_enable[i] = 0
        u.enable_input(InpSel.SRC_0, 1)
        u.enable_input(InpSel.CONST_0, 2)
        u.enable_input(InpSel.SRC_1, 3)
        for p in u.out_enable:
            u.out_enable[p] = 0
        u.enable_output(OutSel.ALU_OUT, OutPath.WR0_LO)
        u.require_inp0 = 1
        u.require_inp1 = 1

        dp = u.datapath_config
        for b in dp:
            b.op = AluOp.BYPASS
            b.alu_src0 = AluInp.PREV_ALU_OUT
            b.alu_src1 = AluInp.PREV_ALU_OUT
            b.alu_out_enable = 1
            b.swap_enable = 0
            b.alu_out_a_enable = 0
            b.alu_out_b_enable = 0
            for c in range(len(b.delay)):
                b.delay[c] = DelayInp.PREV_ALU_OUT
                b.delay_enable[c] = 0

        # blk0: ALU = bypass(x); chain3 <- x (reads as x(i-1) downstream);
        #       carry v1 (0), C0 (1), x (2) onward
        dp[0].enable_alu(AluOp.BYPASS, AluInp.PREV_DELAY_2)
        dp[0].pass_through_delay(0, 1, 2)
        dp[0].enable_delay_from_src(DelayInp.CURR_ALU_OUT, 3)
        # blk1: vm = max(v1(i), x(i)); chain4 <- vm (reads as vm(i-1))
        dp[1].enable_alu(AluOp.MAX, AluInp.PREV_DELAY_0, AluInp.PREV_DELAY_2)
        dp[1].pass_through_delay(1, 3)
        dp[1].enable_delay_from_src(DelayInp.CURR_ALU_OUT, 4)
        # blk2: m1 = max(vm(i), vm(i-1)); chain5 <- m1 (reads as m1(i-1))
        dp[2].enable_alu(AluOp.MAX, AluInp.PREV_ALU_OUT, AluInp.PREV_DELAY_4)
        dp[2].pass_through_delay(1, 3)
        dp[2].enable_delay_from_src(DelayInp.CURR_ALU_OUT, 5)
        # blk3: M = max(m1(i), m1(i-1)) = max(vm(i-2..i))
        dp[3].enable_alu(AluOp.MAX, AluInp.PREV_ALU_OUT, AluInp.PREV_DELAY_5)
        dp[3].pass_through_delay(1, 3)
        # blk4: clamp with C0
        dp[4].enable_alu(AluOp.MAX, AluInp.PREV_ALU_OUT, AluInp.PREV_DELAY_1)
        dp[4].pass_through_delay(3)
        # blk5: out = (Mc <= x(i-1))  i.e. x(i-1) >= window max
        dp[5].enable_alu(AluOp.IS_LE, AluInp.PREV_ALU_OUT, AluInp.PREV_DELAY_3)
        return u


    _READY = {}


    def make_ops(ver="v3"):
        if _READY:
            return _READY["fused"]
        base = lower(Spec(body=maxx(maxx(Src0, DC0), Src1)), ver=ver)
        assert len(base) == 1, len(base)

        fused_spec = Spec(body=maxx(maxx(Src0, DC0), Src1))  # dummy; cache hit

        FUSED = DveOp("ANT_NMS_FUSED", fused_spec, subdim=False, uops_sha={})
        import concourse.dve_ops as dmod
        OPS.append(FUSED)
        for i, op in enumerate(OPS):
            dmod._SUB_OPCODE_FOR_NAME[op.name] = dmod._CUSTOM_DVE_ROW_BASE + i
        dmod.CUSTOM_DVE_SPECS[FUSED.name] = FUSED.spec

        uf = _mk_fused_uop(base[0])

        _COMPILE_CACHE[("ANT_NMS_FUSED", ver)] = DveOpSpec(
            name="ANT_NMS_FUSED", opcode=get_dve_sub_opcode("ANT_NMS_FUSED"),
            uops=[uf], rd1_en=True)
        _READY["fused"] = FUSED
        return FUSED

    FUSED = make_ops()

    # tile-major staged input: for tile t a contiguous [128, 26, WT] block
    XTOT = sum(_tile_geom(t)[3] for t in range(NT)) * (R + 2) * 128
    # tile-major mask out: for tile t a contiguous [128, 24, V+2] block
    MSKW = [WIDTHS[t] + 2 for t in range(NT)]
    MTOT = sum(MSKW) * R * 128

    nc = bacc.Bacc()
    x_in = nc.declare_dram_parameter("x", [XTOT], f32, isOutput=False)
    m_out = nc.declare_dram_parameter("mask", [MTOT], u8, isOutput=True)

    with TileContext(nc) as tc:
        with tc.tile_pool(name="pool", bufs=1) as pool:
            xoff = 0
            moff = 0
            for t in range(NT):
                c0, v, cs, WT = _tile_geom(t)
                a = c0 - cs  # local col offset of the valid range
                WM = MSKW[t]

                xi = bass.AP(x_in, xoff,
                             [[(R + 2) * WT, 128], [WT, R + 2], [1, WT]])
                xoff += 128 * (R + 2) * WT

                X = pool.tile([128, R + 2, WT], f32, tag="X", bufs=3,
                              name=f"X_{t}")
                V1 = pool.tile([128, R, WT], f32, tag="V1", bufs=2,
                               name=f"V1_{t}")
                MSK = pool.tile([128, R, WM], u8, tag="MSK", bufs=2,
                                name=f"MSK_{t}")

                nc.sync.dma_start(out=X[:, :, :], in_=xi)

                # Vertical pair max of the two outer rows. (The gpsimd Pool
                # engine cannot run TT max in this toolchain: walrus codegen
                # only accepts Add/Multiply there.)
                nc.vector.tensor_tensor(
                    V1[:, :, :], X[:, 0:R, :], X[:, 2:R + 2, :], MAX)

                # DVE: fused merge + horizontal sliding max3 + clamp +
                # compare, row-major streams. Junk in the first 2 cols of
                # each row lands in discarded scratch cols (or border
                # cols 0,1 for the first tile).
                if t == 0:
                    # out col k = mask col k; window centered k. MSK cols
                    # v..v+1 stay junk; host reads [0:v].
                    nc.vector._custom_dve(
                        FUSED,
                        out=MSK[:, :, 0:v],
                        in0=V1[:, :, 1:v + 1],
                        in1=X[:, 1:R + 1, 1:v + 1],
                        s0=REP_THR)
                else:
                    # out col k = mask col c0-2+k; valid k in [2, v+2); host
                    # reads [2:v+2]. On the last tile the final column's
                    # window would read past the image edge: shorten the
                    # stream by one; mask col W-1 junk is border, host-zeroed.
                    SL = v + 2 if t < NT - 1 else v + 1
                    nc.vector._custom_dve(
                        FUSED,
                        out=MSK[:, :, 0:SL],
                        in0=V1[:, :, a - 1:a - 1 + SL],
                        in1=X[:, 1:R + 1, a - 1:a - 1 + SL],
                        s0=REP_THR)
                # Mask writes go out on the scalar engine's DMA queue so
                # they never head-of-line-block the input stream on SP.
                mo = bass.AP(m_out, moff, [[R * WM, 128], [1, R * WM]])
                moff += 128 * R * WM
                nc.scalar.dma_start(out=mo, in_=MSK[:, :, :])
    nc.finalize()
    return nc


def _get_program():
    if "nc" not in _CACHE:
        _CACHE["nc"] = _build_program()
    return _CACHE["nc"]


def kernel(repeatability):
    global LAST_RESULTS
    from concourse.bass_utils import run_bass_kernel_spmd

    x = np.asarray(repeatability, dtype=np.float32).reshape(B, H, W)
    xp = np.zeros((B, HP, W), dtype=np.float32)
    xp[:, 1:H + 1, :] = x
    # overlapping row blocks: [B, NB, R+2, W]; block b covers padded rows
    # b*R .. b*R+R+1 (= image rows b*R-1 .. b*R+R)
    st = xp.strides
    xb = np.lib.stride_tricks.as_strided(
        xp, shape=(B, NB, R + 2, W), strides=(st[0], R * st[1], st[1], st[2]))
    xb = xb.reshape(N_CORES, B_PER * NB, R + 2, W)

    # stage tile-major: per core, concat per-tile [128, 26, WT] blocks
    in_maps = []
    for i in range(N_CORES):
        parts = []
        for t in range(NT):
            _, _, cs, WT = _tile_geom(t)
            parts.append(
                np.ascontiguousarray(xb[i, :, :, cs:cs + WT]).reshape(-1))
        in_maps.append({"x": np.concatenate(parts)})

    nc = _get_program()
    res = run_bass_kernel_spmd(nc, in_maps, list(range(N_CORES)),
                               trace=bool(os.environ.get("NMS_TRACE")))
    LAST_RESULTS = res

    # reassemble masks: per tile t the block is [128, 24, V+2]; valid cols
    # are [0:v] for t=0 else [2:v+2]
    mask_full = np.empty((N_CORES, 128, R, W), dtype=np.uint8)
    for i in range(N_CORES):
        flat = res.results[i]["mask"]
        off = 0
        for t in range(NT):
            c0, v = C0[t], WIDTHS[t]
            wm = v + 2
            blk = flat[off:off + 128 * R * wm].reshape(128, R, wm)
            off += 128 * R * wm
            sl = blk[:, :, 0:v] if t == 0 else blk[:, :, 2:v + 2]
            mask_full[i, :, :, c0:c0 + v] = sl
    mask_full = mask_full.reshape(B, C, H, W) != 0
    mask_full[:, :, :10, :] = False
    mask_full[:, :, -10:, :] = False
    mask_full[:, :, :, :10] = False
    mask_full[:, :, :, -10:] = False
    _, _, ys, xs = np.nonzero(mask_full)
    return np.stack([ys, xs]).astype(np.int32)


# revision 22
# speedup vs baseline: 1.1658x; 1.1658x over previous
"""3x3 NMS (maxpool + threshold + border) kernel for Trainium2, 8 NeuronCores.

Strategy:
  - Pure data parallel: 16 images -> 2 images per core on 8 cores.
  - Both images of a core are packed into the partition dim: partition
    p = img*64 + blk holds R=24 image rows (+1 halo row each side, from
    a host-zero-padded copy), so the row-halo DMA overhead is 26/24.
  - The image is split into NT column tiles (2-col halos). The first and
    last tiles are narrow to shrink the pipeline fill / drain on the
    critical path.
  - Host stages the input TILE-MAJOR: for each tile, a contiguous
    [128, 26, WT] block (halo rows/cols duplicated), so every DMA
    descriptor is one full partition worth (26*WT*4 B) and the 16 DMA
    engines run at full rate. The mask output is likewise tile-major
    [128, 24, V+2] contiguous; host reassembles/strips junk columns.
  - Per tile, 2 vector-engine passes:
      1. v1 = max(x_up, x_dn)            (stock tensor_tensor, 1 el/cyc)
      2. mask = ANT_NMS_FUSED(v1, x): a custom DVE uop computing
         vm = max(v1, x), the horizontal sliding 3-max of vm, the 0.6
         clamp, and the compare  mask[c] = (x[c] >= max(vm[c-1..c+1],
         0.6))  in ONE 1-elem/cycle pass, using delay-chain captures for
         the previous-element taps (incl. a 1-delayed x for the center).
    x >= max(window incl. center, thr) is exactly
    (x == maxpool3x3(x)) & (x >= thr): bit-exact, no FP hazards.
  - Host: zero 10px border, np.nonzero -> (y, x) rows, exactly matching
    jnp.nonzero order (batch-major, then row, then col).
"""

import os
import sys

sys.path.insert(0, "/opt/trn_rl_repo")

import numpy as np

B, C, H, W = 16, 1, 1536, 1536
HP = H + 2                    # padded rows
N_CORES = 8
B_PER = B // N_CORES          # images per core
R = 24                        # rows per partition (2 imgs * 64 blocks = 128)
NB = H // R                   # row blocks per image (64)
PAD = 2                       # column halo on each side
REP_THR = 0.6

# column tile widths: progressive ramp so the DVE can start while the DMA
# stream is still ramping, without ever outrunning it
WIDTHS = [32, 128, 192, 240, 240, 240, 240, 224]
assert sum(WIDTHS) == W
NT = len(WIDTHS)
# tile t covers mask cols [C0[t], C0[t]+WIDTHS[t]), reads [cs, ce)
C0 = [sum(WIDTHS[:i]) for i in range(NT)]

_CACHE = {}
LAST_RESULTS = None


def _tile_geom(t):
    c0, v = C0[t], WIDTHS[t]
    cs = max(c0 - PAD, 0)
    ce = min(c0 + v + PAD, W)
    return c0, v, cs, ce - cs


def _build_program():
    import concourse.bass as bass
    import concourse.bacc as bacc
    import concourse.mybir as mybir
    from concourse.tile import TileContext

    f32 = mybir.dt.float32
    u8 = mybir.dt.uint8
    MAX = mybir.AluOpType.max

    from concourse.dve_ops import DveOp, OPS, _COMPILE_CACHE
    from concourse.dve_spec import Spec, Src0, Src1, C0 as DC0, maxx, lower
    from concourse.dve_uop import (
        DveOpSpec, InpSel, OutSel, OutPath, AluInp, DelayInp, AluOp,
    )
    from concourse.dve_ops import get_dve_sub_opcode


    def _mk_fused_uop(base_uop):
        """One fused NMS pass. Stream pos i carries v1[i] (src0) and x[i]
        (src1); output at pos i is mask for the column one behind:
        out(i) = (x(i-1) >= max(0.6, vm(i-2), vm(i-1), vm(i))) with
        vm(j) = max(v1(j), x(j)).

        Delay chains (v3 has 6): 0 = v1 in, 1 = C0, 2 = x in,
        3 = x delayed one element, 4 = vm(i-1) tap, 5 = m1(i-1) tap.
        """
        u = base_uop  # copy of a lowered stock uop: keeps FSM/trigger/ctrl
        for i in range(len(u.inp)):
            u.inp_enable[i] = 0
        u.enable_input(InpSel.SRC_0, 1)
        u.enable_input(InpSel.CONST_0, 2)
        u.enable_input(InpSel.SRC_1, 3)
        for p in u.out_enable:
            u.out_enable[p] = 0
        u.enable_output(OutSel.ALU_OUT, OutPath.WR0_LO)
        u.require_inp0 = 1
        u.require_inp1 = 1

        dp = u.datapath_config
        for b in dp:
            b.op = AluOp.BYPASS
            b.alu_src0 = AluInp.PREV_ALU_OUT
            b.alu_src1 = AluInp.PREV_ALU_OUT
            b.alu_out_enable = 1
            b.swap_enable = 0
            b.alu_out_a_enable = 0
            b.alu_out_b_enable = 0
            for c in range(len(b.delay)):
                b.delay[c] = DelayInp.PREV_ALU_OUT
                b.delay_enable[c] = 0

        # blk0: ALU = bypass(x); chain3 <- x (reads as x(i-1) downstream);
        #       carry v1 (0), C0 (1), x (2) onward
        dp[0].enable_alu(AluOp.BYPASS, AluInp.PREV_DELAY_2)
        dp[0].pass_through_delay(0, 1, 2)
        dp[0].enable_delay_from_src(DelayInp.CURR_ALU_OUT, 3)
        # blk1: vm = max(v1(i), x(i)); chain4 <- vm (reads as vm(i-1))
        dp[1].enable_alu(AluOp.MAX, AluInp.PREV_DELAY_0, AluInp.PREV_DELAY_2)
        dp[1].pass_through_delay(1, 3)
        dp[1].enable_delay_from_src(DelayInp.CURR_ALU_OUT, 4)
        # blk2: m1 = max(vm(i), vm(i-1)); chain5 <- m1 (reads as m1(i-1))
        dp[2].enable_alu(AluOp.MAX, AluInp.PREV_ALU_OUT, AluInp.PREV_DELAY_4)
        dp[2].pass_through_delay(1, 3)
        dp[2].enable_delay_from_src(DelayInp.CURR_ALU_OUT, 5)
        # blk3: M = max(m1(i), m1(i-1)) = max(vm(i-2..i))
        dp[3].enable_alu(AluOp.MAX, AluInp.PREV_ALU_OUT, AluInp.PREV_DELAY_5)
        dp[3].pass_through_delay(1, 3)
        # blk4: clamp with C0
        dp[4].enable_alu(AluOp.MAX, AluInp.PREV_ALU_OUT, AluInp.PREV_DELAY_1)
        dp[4].pass_through_delay(3)
        # blk5: out = (Mc <= x(i-1))  i.e. x(i-1) >= window max
        dp[5].enable_alu(AluOp.IS_LE, AluInp.PREV_ALU_OUT, AluInp.PREV_DELAY_3)
        return u


    _READY = {}


    def make_ops(ver="v3"):
        if _READY:
            return _READY["fused"]
        base = lower(Spec(body=maxx(maxx(Src0, DC0), Src1)), ver=ver)
        assert len(base) == 1, len(base)

        fused_spec = Spec(body=maxx(maxx(Src0, DC0), Src1))  # dummy; cache hit

        FUSED = DveOp("ANT_NMS_FUSED", fused_spec, subdim=False, uops_sha={})
        import concourse.dve_ops as dmod
        OPS.append(FUSED)
        for i, op in enumerate(OPS):
            dmod._SUB_OPCODE_FOR_NAME[op.name] = dmod._CUSTOM_DVE_ROW_BASE + i
        dmod.CUSTOM_DVE_SPECS[FUSED.name] = FUSED.spec

        uf = _mk_fused_uop(base[0])

        _COMPILE_CACHE[("ANT_NMS_FUSED", ver)] = DveOpSpec(
            name="ANT_NMS_FUSED", opcode=get_dve_sub_opcode("ANT_NMS_FUSED"),
            uops=[uf], rd1_en=True)
        _READY["fused"] = FUSED
        return FUSED

    FUSED = make_ops()

    # tile-major staged input: for tile t a contiguous [128, 26, WT] block
    XTOT = sum(_tile_geom(t)[3] for t in range(NT)) * (R + 2) * 128
    # tile-major mask out: for tile t a contiguous [128, 24, V+2] block
    MSKW = [WIDTHS[t] + 2 for t in range(NT)]
    MTOT = sum(MSKW) * R * 128

    nc = bacc.Bacc()
    x_in = nc.declare_dram_parameter("x", [XTOT], f32, isOutput=False)
    m_out = nc.declare_dram_parameter("mask", [MTOT], u8, isOutput=True)

    with TileContext(nc) as tc:
        with tc.tile_pool(name="pool", bufs=1) as pool:
            xoff = 0
            moff = 0
            for t in range(NT):
                c0, v, cs, WT = _tile_geom(t)
                a = c0 - cs  # local col offset of the valid range
                WM = MSKW[t]

                xi = bass.AP(x_in, xoff,
                             [[(R + 2) * WT, 128], [WT, R + 2], [1, WT]])
                xoff += 128 * (R + 2) * WT

                X = pool.tile([128, R + 2, WT], f32, tag="X", bufs=2,
                              name=f"X_{t}")
                V1 = pool.tile([128, R, WT], f32, tag="V1", bufs=2,
                               name=f"V1_{t}")
                MSK = pool.tile([128, R, WM], u8, tag="MSK", bufs=2,
                                name=f"MSK_{t}")

                nc.sync.dma_start(out=X[:, :, :], in_=xi)

                # Vertical pair max of the two outer rows. (The gpsimd Pool
                # engine cannot run TT max in this toolchain: walrus codegen
                # only accepts Add/Multiply there.)
                nc.vector.tensor_tensor(
                    V1[:, :, :], X[:, 0:R, :], X[:, 2:R + 2, :], MAX)

                # DVE: fused merge + horizontal sliding max3 + clamp +
                # compare, row-major streams. Junk in the first 2 cols of
                # each row lands in discarded scratch cols (or border
                # cols 0,1 for the first tile).
                if t == 0:
                    # out col k = mask col k; window centered k. MSK cols
                    # v..v+1 stay junk; host reads [0:v].
                    nc.vector._custom_dve(
                        FUSED,
                        out=MSK[:, :, 0:v],
                        in0=V1[:, :, 1:v + 1],
                        in1=X[:, 1:R + 1, 1:v + 1],
                        s0=REP_THR)
                else:
                    # out col k = mask col c0-2+k; valid k in [2, v+2); host
                    # reads [2:v+2]. On the last tile the final column's
                    # window would read past the image edge: shorten the
                    # stream by one; mask col W-1 junk is border, host-zeroed.
                    SL = v + 2 if t < NT - 1 else v + 1
                    nc.vector._custom_dve(
                        FUSED,
                        out=MSK[:, :, 0:SL],
                        in0=V1[:, :, a - 1:a - 1 + SL],
                        in1=X[:, 1:R + 1, a - 1:a - 1 + SL],
                        s0=REP_THR)
                # Mask writes go out on the scalar engine's DMA queue so
                # they never head-of-line-block the input stream on SP.
                mo = bass.AP(m_out, moff, [[R * WM, 128], [1, R * WM]])
                moff += 128 * R * WM
                nc.scalar.dma_start(out=mo, in_=MSK[:, :, :])
    nc.finalize()
    return nc


def _get_program():
    if "nc" not in _CACHE:
        _CACHE["nc"] = _build_program()
    return _CACHE["nc"]


def kernel(repeatability):
    global LAST_RESULTS
    from concourse.bass_utils import run_bass_kernel_spmd

    x = np.asarray(repeatability, dtype=np.float32).reshape(B, H, W)
    xp = np.zeros((B, HP, W), dtype=np.float32)
    xp[:, 1:H + 1, :] = x
    # overlapping row blocks: [B, NB, R+2, W]; block b covers padded rows
    # b*R .. b*R+R+1 (= image rows b*R-1 .. b*R+R)
    st = xp.strides
    xb = np.lib.stride_tricks.as_strided(
        xp, shape=(B, NB, R + 2, W), strides=(st[0], R * st[1], st[1], st[2]))
    xb = xb.reshape(N_CORES, B_PER * NB, R + 2, W)

    # stage tile-major: per core, concat per-tile [128, 26, WT] blocks
    in_maps = []
    for i in range(N_CORES):
        parts = []
        for t in range(NT):
            _, _, cs, WT = _tile_geom(t)
            parts.append(
                np.ascontiguousarray(xb[i, :, :, cs:cs + WT]).reshape(-1))
        in_maps.append({"x": np.concatenate(parts)})

    nc = _get_program()
    res = run_bass_kernel_spmd(nc, in_maps, list(range(N_CORES)),
                               trace=bool(os.environ.get("NMS_TRACE")))
    LAST_RESULTS = res

    # reassemble masks: per tile t the block is [128, 24, V+2]; valid cols
    # are [0:v] for t=0 else [2:v+2]
    mask_full = np.empty((N_CORES, 128, R, W), dtype=np.uint8)
    for i in range(N_CORES):
        flat = res.results[i]["mask"]
        off = 0
        for t in range(NT):
            c0, v = C0[t], WIDTHS[t]
            wm = v + 2
            blk = flat[off:off + 128 * R * wm].reshape(128, R, wm)
            off += 128 * R * wm
            sl = blk[:, :, 0:v] if t == 0 else blk[:, :, 2:v + 2]
            mask_full[i, :, :, c0:c0 + v] = sl
    mask_full = mask_full.reshape(B, C, H, W) != 0
    mask_full[:, :, :10, :] = False
    mask_full[:, :, -10:, :] = False
    mask_full[:, :, :, :10] = False
    mask_full[:, :, :, -10:] = False
    _, _, ys, xs = np.nonzero(mask_full)
    return np.stack([ys, xs]).astype(np.int32)


# revision 23
# speedup vs baseline: 1.2152x; 1.0424x over previous
"""3x3 NMS (maxpool + threshold + border) kernel for Trainium2, 8 NeuronCores.

Strategy:
  - Pure data parallel: 16 images -> 2 images per core on 8 cores.
  - Both images of a core are packed into the partition dim: partition
    p = img*64 + blk holds R=24 image rows (+1 halo row each side, from
    a host-zero-padded copy), so the row-halo DMA overhead is 26/24.
  - The image is split into NT column tiles (2-col halos). The first and
    last tiles are narrow to shrink the pipeline fill / drain on the
    critical path.
  - Host stages the input TILE-MAJOR: for each tile, a contiguous
    [128, 26, WT] block (halo rows/cols duplicated), so every DMA
    descriptor is one full partition worth (26*WT*4 B) and the 16 DMA
    engines run at full rate. The mask output is likewise tile-major
    [128, 24, V+2] contiguous; host reassembles/strips junk columns.
  - Per tile, 2 vector-engine passes:
      1. v1 = max(x_up, x_dn)            (stock tensor_tensor, 1 el/cyc)
      2. mask = ANT_NMS_FUSED(v1, x): a custom DVE uop computing
         vm = max(v1, x), the horizontal sliding 3-max of vm, the 0.6
         clamp, and the compare  mask[c] = (x[c] >= max(vm[c-1..c+1],
         0.6))  in ONE 1-elem/cycle pass, using delay-chain captures for
         the previous-element taps (incl. a 1-delayed x for the center).
    x >= max(window incl. center, thr) is exactly
    (x == maxpool3x3(x)) & (x >= thr): bit-exact, no FP hazards.
  - Host: zero 10px border, np.nonzero -> (y, x) rows, exactly matching
    jnp.nonzero order (batch-major, then row, then col).
"""

import os
import sys

sys.path.insert(0, "/opt/trn_rl_repo")

import numpy as np

B, C, H, W = 16, 1, 1536, 1536
HP = H + 2                    # padded rows
N_CORES = 8
B_PER = B // N_CORES          # images per core
R = 24                        # rows per partition (2 imgs * 64 blocks = 128)
NB = H // R                   # row blocks per image (64)
PAD = 2                       # column halo on each side
REP_THR = 0.6

# column tile widths: progressive ramp so the DVE can start while the DMA
# stream is still ramping, without ever outrunning it
WIDTHS = [32, 128, 192, 240, 240, 240, 240, 224]
assert sum(WIDTHS) == W
NT = len(WIDTHS)
# tile t covers mask cols [C0[t], C0[t]+WIDTHS[t]), reads [cs, ce)
C0 = [sum(WIDTHS[:i]) for i in range(NT)]

_CACHE = {}
LAST_RESULTS = None


def _tile_geom(t):
    c0, v = C0[t], WIDTHS[t]
    cs = max(c0 - PAD, 0)
    ce = min(c0 + v + PAD, W)
    return c0, v, cs, ce - cs


def _build_program():
    import concourse.bass as bass
    import concourse.bacc as bacc
    import concourse.mybir as mybir
    from concourse.tile import TileContext

    f32 = mybir.dt.float32
    u8 = mybir.dt.uint8
    MAX = mybir.AluOpType.max

    from concourse.dve_ops import DveOp, OPS, _COMPILE_CACHE
    from concourse.dve_spec import Spec, Src0, Src1, C0 as DC0, maxx, lower
    from concourse.dve_uop import (
        DveOpSpec, InpSel, OutSel, OutPath, AluInp, DelayInp, AluOp,
    )
    from concourse.dve_ops import get_dve_sub_opcode


    def _mk_fused_uop(base_uop):
        """One fused NMS pass. Stream pos i carries v1[i] (src0) and x[i]
        (src1); output at pos i is mask for the column one behind:
        out(i) = (x(i-1) >= max(0.6, vm(i-2), vm(i-1), vm(i))) with
        vm(j) = max(v1(j), x(j)).

        Delay chains (v3 has 6): 0 = v1 in, 1 = C0, 2 = x in,
        3 = x delayed one element, 4 = vm(i-1) tap, 5 = m1(i-1) tap.
        """
        u = base_uop  # copy of a lowered stock uop: keeps FSM/trigger/ctrl
        for i in range(len(u.inp)):
            u.inp_enable[i] = 0
        u.enable_input(InpSel.SRC_0, 1)
        u.enable_input(InpSel.CONST_0, 2)
        u.enable_input(InpSel.SRC_1, 3)
        for p in u.out_enable:
            u.out_enable[p] = 0
        u.enable_output(OutSel.ALU_OUT, OutPath.WR0_LO)
        u.require_inp0 = 1
        u.require_inp1 = 1

        dp = u.datapath_config
        for b in dp:
            b.op = AluOp.BYPASS
            b.alu_src0 = AluInp.PREV_ALU_OUT
            b.alu_src1 = AluInp.PREV_ALU_OUT
            b.alu_out_enable = 1
            b.swap_enable = 0
            b.alu_out_a_enable = 0
            b.alu_out_b_enable = 0
            for c in range(len(b.delay)):
                b.delay[c] = DelayInp.PREV_ALU_OUT
                b.delay_enable[c] = 0

        # blk0: ALU = bypass(x); chain3 <- x (reads as x(i-1) downstream);
        #       carry v1 (0), C0 (1), x (2) onward
        dp[0].enable_alu(AluOp.BYPASS, AluInp.PREV_DELAY_2)
        dp[0].pass_through_delay(0, 1, 2)
        dp[0].enable_delay_from_src(DelayInp.CURR_ALU_OUT, 3)
        # blk1: vm = max(v1(i), x(i)); chain4 <- vm (reads as vm(i-1))
        dp[1].enable_alu(AluOp.MAX, AluInp.PREV_DELAY_0, AluInp.PREV_DELAY_2)
        dp[1].pass_through_delay(1, 3)
        dp[1].enable_delay_from_src(DelayInp.CURR_ALU_OUT, 4)
        # blk2: m1 = max(vm(i), vm(i-1)); chain5 <- m1 (reads as m1(i-1))
        dp[2].enable_alu(AluOp.MAX, AluInp.PREV_ALU_OUT, AluInp.PREV_DELAY_4)
        dp[2].pass_through_delay(1, 3)
        dp[2].enable_delay_from_src(DelayInp.CURR_ALU_OUT, 5)
        # blk3: M = max(m1(i), m1(i-1)) = max(vm(i-2..i))
        dp[3].enable_alu(AluOp.MAX, AluInp.PREV_ALU_OUT, AluInp.PREV_DELAY_5)
        dp[3].pass_through_delay(1, 3)
        # blk4: clamp with C0
        dp[4].enable_alu(AluOp.MAX, AluInp.PREV_ALU_OUT, AluInp.PREV_DELAY_1)
        dp[4].pass_through_delay(3)
        # blk5: out = (Mc <= x(i-1))  i.e. x(i-1) >= window max
        dp[5].enable_alu(AluOp.IS_LE, AluInp.PREV_ALU_OUT, AluInp.PREV_DELAY_3)
        return u


    _READY = {}


    def make_ops(ver="v3"):
        if _READY:
            return _READY["fused"]
        base = lower(Spec(body=maxx(maxx(Src0, DC0), Src1)), ver=ver)
        assert len(base) == 1, len(base)

        fused_spec = Spec(body=maxx(maxx(Src0, DC0), Src1))  # dummy; cache hit

        FUSED = DveOp("ANT_NMS_FUSED", fused_spec, subdim=False, uops_sha={})
        import concourse.dve_ops as dmod
        OPS.append(FUSED)
        for i, op in enumerate(OPS):
            dmod._SUB_OPCODE_FOR_NAME[op.name] = dmod._CUSTOM_DVE_ROW_BASE + i
        dmod.CUSTOM_DVE_SPECS[FUSED.name] = FUSED.spec

        uf = _mk_fused_uop(base[0])

        _COMPILE_CACHE[("ANT_NMS_FUSED", ver)] = DveOpSpec(
            name="ANT_NMS_FUSED", opcode=get_dve_sub_opcode("ANT_NMS_FUSED"),
            uops=[uf], rd1_en=True)
        _READY["fused"] = FUSED
        return FUSED

    FUSED = make_ops()

    # tile-major staged input: for tile t a contiguous [128, 26, WT] block
    XTOT = sum(_tile_geom(t)[3] for t in range(NT)) * (R + 2) * 128
    # tile-major mask out: for tile t a contiguous [128, 24, V+2] block
    MSKW = [WIDTHS[t] + 2 for t in range(NT)]
    MTOT = sum(MSKW) * R * 128

    nc = bacc.Bacc()
    x_in = nc.declare_dram_parameter("x", [XTOT], f32, isOutput=False)
    m_out = nc.declare_dram_parameter("mask", [MTOT], u8, isOutput=True)

    with TileContext(nc) as tc:
        with tc.tile_pool(name="pool", bufs=1) as pool:
            xoff = 0
            moff = 0
            for t in range(NT):
                c0, v, cs, WT = _tile_geom(t)
                a = c0 - cs  # local col offset of the valid range
                WM = MSKW[t]

                xi = bass.AP(x_in, xoff,
                             [[(R + 2) * WT, 128], [WT, R + 2], [1, WT]])
                xoff += 128 * (R + 2) * WT

                X = pool.tile([128, R + 2, WT], f32, tag="X", bufs=3,
                              name=f"X_{t}")
                V1 = pool.tile([128, R, WT], f32, tag="V1", bufs=2,
                               name=f"V1_{t}")
                MSK = pool.tile([128, R, WM], u8, tag="MSK", bufs=2,
                                name=f"MSK_{t}")

                nc.sync.dma_start(out=X[:, :, :], in_=xi)

                # Vertical pair max of the two outer rows. (The gpsimd Pool
                # engine cannot run TT max in this toolchain: walrus codegen
                # only accepts Add/Multiply there.)
                nc.vector.tensor_tensor(
                    V1[:, :, :], X[:, 0:R, :], X[:, 2:R + 2, :], MAX)

                # DVE: fused merge + horizontal sliding max3 + clamp +
                # compare, row-major streams. Junk in the first 2 cols of
                # each row lands in discarded scratch cols (or border
                # cols 0,1 for the first tile).
                if t == 0:
                    # out col k = mask col k; window centered k. MSK cols
                    # v..v+1 stay junk; host reads [0:v].
                    nc.vector._custom_dve(
                        FUSED,
                        out=MSK[:, :, 0:v],
                        in0=V1[:, :, 1:v + 1],
                        in1=X[:, 1:R + 1, 1:v + 1],
                        s0=REP_THR)
                else:
                    # out col k = mask col c0-2+k; valid k in [2, v+2); host
                    # reads [2:v+2]. On the last tile the final column's
                    # window would read past the image edge: shorten the
                    # stream by one; mask col W-1 junk is border, host-zeroed.
                    SL = v + 2 if t < NT - 1 else v + 1
                    nc.vector._custom_dve(
                        FUSED,
                        out=MSK[:, :, 0:SL],
                        in0=V1[:, :, a - 1:a - 1 + SL],
                        in1=X[:, 1:R + 1, a - 1:a - 1 + SL],
                        s0=REP_THR)
                # Mask writes go out on the scalar engine's DMA queue so
                # they never head-of-line-block the input stream on SP.
                mo = bass.AP(m_out, moff, [[R * WM, 128], [1, R * WM]])
                moff += 128 * R * WM
                nc.scalar.dma_start(out=mo, in_=MSK[:, :, :])
    nc.finalize()
    return nc


def _get_program():
    if "nc" not in _CACHE:
        _CACHE["nc"] = _build_program()
    return _CACHE["nc"]


def kernel(repeatability):
    global LAST_RESULTS
    from concourse.bass_utils import run_bass_kernel_spmd

    x = np.asarray(repeatability, dtype=np.float32).reshape(B, H, W)
    xp = np.zeros((B, HP, W), dtype=np.float32)
    xp[:, 1:H + 1, :] = x
    # overlapping row blocks: [B, NB, R+2, W]; block b covers padded rows
    # b*R .. b*R+R+1 (= image rows b*R-1 .. b*R+R)
    st = xp.strides
    xb = np.lib.stride_tricks.as_strided(
        xp, shape=(B, NB, R + 2, W), strides=(st[0], R * st[1], st[1], st[2]))
    xb = xb.reshape(N_CORES, B_PER * NB, R + 2, W)

    # stage tile-major: per core, concat per-tile [128, 26, WT] blocks
    in_maps = []
    for i in range(N_CORES):
        parts = []
        for t in range(NT):
            _, _, cs, WT = _tile_geom(t)
            parts.append(
                np.ascontiguousarray(xb[i, :, :, cs:cs + WT]).reshape(-1))
        in_maps.append({"x": np.concatenate(parts)})

    nc = _get_program()
    res = run_bass_kernel_spmd(nc, in_maps, list(range(N_CORES)),
                               trace=bool(os.environ.get("NMS_TRACE")))
    LAST_RESULTS = res

    # reassemble masks: per tile t the block is [128, 24, V+2]; valid cols
    # are [0:v] for t=0 else [2:v+2]
    mask_full = np.empty((N_CORES, 128, R, W), dtype=np.uint8)
    for i in range(N_CORES):
        flat = res.results[i]["mask"]
        off = 0
        for t in range(NT):
            c0, v = C0[t], WIDTHS[t]
            wm = v + 2
            blk = flat[off:off + 128 * R * wm].reshape(128, R, wm)
            off += 128 * R * wm
            sl = blk[:, :, 0:v] if t == 0 else blk[:, :, 2:v + 2]
            mask_full[i, :, :, c0:c0 + v] = sl
    mask_full = mask_full.reshape(B, C, H, W) != 0
    mask_full[:, :, :10, :] = False
    mask_full[:, :, -10:, :] = False
    mask_full[:, :, :, :10] = False
    mask_full[:, :, :, -10:] = False
    _, _, ys, xs = np.nonzero(mask_full)
    return np.stack([ys, xs]).astype(np.int32)
